# revision 6
# baseline (speedup 1.0000x reference)
"""CrossScaleAttention Trainium2 kernel (v2).

Full (unsharded) contract: kernel(query, key, value) with shapes
  query/key/value: (4, 4096, 256) float32  ->  out (4, 4096, 256) float32

reference math:
  q = l2norm(query); k = l2norm(key)
  out = softmax((q @ k^T) * 32**-0.5) @ value

Sharding: 8 cores; core c computes batch c//2, query rows (c%2)*2048..+2048,
with that batch's full K/V resident per core (no collectives needed).

v2 changes over the 150us baseline (bottlenecks measured from its NTFF):
  - ACT exp ran at (N+352)/1.2 ns per ACTIVATE: 128x 512-wide exps = 92us,
    co-limiting with the PE. Now BOTH q and k rows are pre-normalized to
    unit length on DVE before fp8 quantization, so exp's scale becomes the
    global constant 32**-0.5 instead of a per-key AP, and one ACTIVATE can
    span several PSUM banks: blocks 1-3 use [128, 2, 512] score groups ->
    64 ACTIVATEs of 1024 -> ACT drops below the PE roofline.
  - inputs DMA at ~350 GB/s; the baseline PE idled 21.6us at the start on
    coarse, badly ordered input DMAs. Now DMAs issue in exact consumption
    order in ~512KB pieces and the k/q pipeline (norm -> unit-scale+bf16
    cast -> PE transpose -> fp8 copy) chases each arrival.
  - PE transposes ran two-pass fp32 (~272ns each): operands now cast to
    bf16 before the transpose (~90ns), and V DMAs straight into the f32r
    AV operand (f32r is bit-identical fp32; the PE reads it reduced),
    deleting all V staging copies.
  - 48 dummy 64x64 matmuls at t=0 warm the PE HAM clock gate (cold default
    is K=4/8 = 1.2 GHz) during the DMA dead time, so real matmuls run at
    2.4 GHz from the first chunk; the baseline spent its first 26.7us cold.

Per-core algorithm (all PE matmuls contract over the partition dim):
  - S^T chunks [128 keys, 512 queries]: K^T/Q^T stored fp8e4m3 unit-norm,
    one DoubleRow matmul per chunk contracts all 256 d at 2 MACs/cell/cyc.
  - exp(SCALE * s) via ACT PSUM->SBUF producing P^T in f32r. No
    max-subtraction needed: |logit| <= 0.177, exp cannot overflow.
  - AV (f32r): out_psum[128 q, 258] += P^T_chunk.T @ [V | 1 1]; the ones
    columns accumulate the softmax denominator in the same chain.
  - epilogue: out = out_psum[:, :256] * (1 / out_psum[:, 256]).
"""

import sys

if "/opt/trn_rl_repo" not in sys.path:
    sys.path.insert(0, "/opt/trn_rl_repo")

import numpy as np

import concourse.bass as bass
import concourse.mybir as mybir
import concourse.tile as tile
from concourse import bacc
from concourse.bass_utils import run_bass_kernel_spmd
from concourse.masks import make_identity

F32 = mybir.dt.float32
F32R = mybir.dt.float32r
BF16 = mybir.dt.bfloat16
FP8 = mybir.dt.float8e4
I32 = mybir.dt.int32

B, NQ_FULL, NK, D = 4, 4096, 4096, 256
N_CORES = 8
NQ = NQ_FULL * B // N_CORES  # 2048 queries per core
P = 128
DC = D // P          # 2 d-chunks
KC = NK // P         # 32 key chunks
QTI = NQ // P        # 16 q tiles
QB = 512             # queries per block
NB = NQ // QB        # 4 blocks
QT = QB // P         # 4 q-subtiles per block
VW = D + 2           # V columns padded with two 1.0 columns (even moving-dim)
NT = KC + QTI        # 48 row tiles total
SCALE = float(D // 8) ** -0.5  # head_dim**-0.5 = 32**-0.5
RSQRT_MAGIC = 0x5F3759DF
N_WARM = 48          # HAM warm-up matmuls at t=0

Exp = mybir.ActivationFunctionType.Exp
DRow = mybir.MatmulPerfMode.DoubleRow

# natall row positions: q0-3 -> 0..3, k0-31 -> 4..35, q4-15 -> 36..47
QPOS0, KPOS, QPOS1 = 0, 4, 36


def _build_program():
    nc = bacc.Bacc(
        "TRN2",
        target_bir_lowering=False,
        debug=False,
        enable_asserts=False,
        num_devices=N_CORES,
    )
    q_d = nc.dram_tensor("q", (NQ, D), F32, kind="ExternalInput").ap()
    k_d = nc.dram_tensor("k", (NK, D), F32, kind="ExternalInput").ap()
    v_d = nc.dram_tensor("v", (NK, D), F32, kind="ExternalInput").ap()
    o_d = nc.dram_tensor("o", (NQ, D), F32, kind="ExternalOutput").ap()

    k_re = k_d.rearrange("(i p) d -> p i d", p=P)  # [128, 32, 256]
    q_re = q_d.rearrange("(i p) d -> p i d", p=P)  # [128, 16, 256]
    v_re = v_d.rearrange("(i p) d -> p i d", p=P)  # [128, 32, 256]

    with tile.TileContext(nc) as tc:
        with (
            tc.tile_pool(name="const", bufs=1) as const_pool,
            tc.tile_pool(name="persist", bufs=1) as persist,
            tc.tile_pool(name="nstage", bufs=3) as nstage,
            tc.tile_pool(name="small", bufs=8) as small,
            tc.tile_pool(name="pt", bufs=3) as pt_pool,
            tc.tile_pool(name="outs", bufs=3) as out_pool,
            tc.tile_pool(name="avps", bufs=1, space="PSUM") as av_pool,
        ):
            identb = const_pool.tile([P, P], BF16)
            make_identity(nc, identb)
            ones = const_pool.tile([P, 1], F32)
            nc.vector.memset(ones, 1.0)
            magic = const_pool.tile([P, 1], I32)
            nc.vector.memset(magic, RSQRT_MAGIC)
            wdat = const_pool.tile([P, 64], BF16)
            nc.vector.memset(wdat, 0.0)

            # persistent operands
            kt = persist.tile([P, DC, NK], FP8)     # K^T [d, keys] unit rows
            qt = persist.tile([P, DC, NQ], FP8)     # Q^T [d, queries] unit rows
            va = persist.tile([P, KC, VW], F32R)    # [keys, d | ones ones]
            natall = persist.tile([P, NT, D], F32)  # raw rows
            ssall = persist.tile([P, NT], F32)      # row sum-of-squares
            rinv_all = persist.tile([P, NT], F32)   # 1 / ||row||

            nc.vector.tensor_copy(
                va[:, :, D:VW], ones[:, :, None].to_broadcast((P, KC, 2))
            )

            # ---- input DMAs, issued in exact consumption order ----
            # (v stages through SBUF f32: a DMA may not produce an f32r
            # matmul operand directly -- the BIR verifier requires a
            # rounding producer, so a DVE copy rounds stage -> va.)
            vstg = []
            nc.sync.dma_start(natall[:, 0:4, :], q_re[:, 0:4, :])       # q0-3
            for g in range(KC // 4):
                nc.sync.dma_start(
                    natall[:, KPOS + 4 * g : KPOS + 4 * g + 4, :],
                    k_re[:, 4 * g : 4 * g + 4, :],
                )
                vs = nstage.tile([P, 4, D], F32, tag="vst", name=f"vst{g}")
                vstg.append(vs)
                nc.sync.dma_start(vs, v_re[:, 4 * g : 4 * g + 4, :])
            nc.sync.dma_start(natall[:, 36:42, :], q_re[:, 4:10, :])    # q4-9
            nc.sync.dma_start(natall[:, 42:48, :], q_re[:, 10:QTI, :])  # q10-15

            def v_copy(g):
                nc.vector.tensor_copy(
                    va[:, 4 * g : 4 * g + 4, 0:D], vstg[g]
                )

            # ---- row norms (all DVE; rsqrt via bit trick + 2 Newton) ----
            def norms(lo, hi):
                n = hi - lo
                sq = nstage.tile([P, n, D], F32, tag="sqg", name=f"sqg{lo}")
                nat = natall[:, lo:hi, :]
                nc.vector.tensor_mul(sq, nat, nat)
                ss = ssall[:, lo:hi]
                nc.vector.tensor_reduce(
                    ss, sq, axis=mybir.AxisListType.X, op=mybir.AluOpType.add
                )
                y = rinv_all[:, lo:hi]
                yi = y.bitcast(I32)
                nc.vector.tensor_scalar(
                    yi, ss.bitcast(I32), 1, None,
                    op0=mybir.AluOpType.logical_shift_right,
                )
                nc.vector.tensor_tensor(
                    yi, magic.to_broadcast((P, n)), yi, mybir.AluOpType.subtract
                )
                t = small.tile([P, n], F32, tag="nt", name=f"nt{lo}")
                for _ in range(2):
                    nc.vector.tensor_mul(t, y, y)
                    nc.vector.tensor_mul(t, t, ss)
                    nc.vector.tensor_scalar(
                        t, t, -0.5, 1.5,
                        op0=mybir.AluOpType.mult, op1=mybir.AluOpType.add,
                    )
                    nc.vector.tensor_mul(y, y, t)

            def unit_cast(lo, hi):
                """bf16 unit rows: natall[lo:hi] * rinv[lo:hi]."""
                n = hi - lo
                ub = nstage.tile([P, n, D], BF16, tag="ub", name=f"ub{lo}")
                nc.vector.tensor_tensor(
                    ub,
                    natall[:, lo:hi, :],
                    rinv_all[:, lo:hi, None].to_broadcast((P, n, D)),
                    mybir.AluOpType.mult,
                )
                return ub

            def new_avs(blk):
                return [
                    av_pool.tile([P, VW], F32, tag=f"av{t}", name=f"av{t}_{blk}")
                    for t in range(QT)
                ]

            def epilogue(blk, avs):
                for t in range(QT):
                    rec = small.tile([P, 1], F32, tag="rec")
                    nc.vector.reciprocal(rec, avs[t][:, D : D + 1])
                    ot = out_pool.tile([P, D], F32, tag="ot")
                    nc.vector.tensor_scalar_mul(ot, avs[t][:, :D], rec)
                    row = blk * QB + t * P
                    nc.sync.dma_start(o_d[row : row + P, :], ot)

            # block 0's accumulators exist before the scoped PSUM pools so
            # pool live ranges stay LIFO
            avs0 = new_avs(0)

            # ================= phase A: prologue + block 0 =================
            with (
                tc.tile_pool(name="ps1", bufs=2, space="PSUM") as ps1,
                tc.tile_pool(name="tpsp", bufs=1, space="PSUM") as tps_pool,
            ):
                # HAM warm-up: dummy matmuls into a corner of avs0[0]; the
                # real AV chain overwrites it with start=True at chunk 0.
                for i in range(N_WARM):
                    nc.tensor.matmul(
                        avs0[0][0:64, 0:64], lhsT=wdat, rhs=wdat,
                        start=True, stop=True,
                    )

                tpst = tps_pool.tile([P, 2, DC, P], BF16)  # ping/pong pairs
                tp_state = [0]

                def transpose_tile(ub, j, idx, dst):
                    """PE-transpose ub[:, j, :] into dst column block idx."""
                    pp = tp_state[0]
                    tp_state[0] ^= 1
                    for dc in range(DC):
                        nc.tensor.transpose(
                            tpst[:, pp, dc, :],
                            ub[:, j, dc * P : (dc + 1) * P],
                            identb,
                        )
                    nc.vector.tensor_copy(
                        dst[:, :, idx * P : (idx + 1) * P], tpst[:, pp]
                    )

                # q0-3 and k0-1 before the first chunk
                norms(0, 4)
                ubq = unit_cast(0, 4)
                for j in range(4):
                    transpose_tile(ubq, j, j, qt)
                norms(KPOS, KPOS + 4)
                ub_hold = [unit_cast(KPOS, KPOS + 4)]
                transpose_tile(ub_hold[0], 0, 0, kt)
                transpose_tile(ub_hold[0], 1, 1, kt)
                v_copy(0)

                def k_norms_cast(g):
                    lo = KPOS + 4 * g
                    norms(lo, lo + 4)
                    ub_hold[0] = unit_cast(lo, lo + 4)

                def k_transposes(a, b, base):
                    for j in range(a, b):
                        transpose_tile(ub_hold[0], j, base + j, kt)

                after = {0: [lambda: k_transposes(2, 4, 0)]}
                for g in range(1, KC // 4):  # groups k4-7 .. k28-31
                    c = 4 * (g - 1)
                    after.setdefault(c, []).append(
                        lambda g=g: k_norms_cast(g)
                    )
                    after.setdefault(c + 1, []).append(
                        lambda g=g: k_transposes(0, 2, 4 * g)
                    )
                    after.setdefault(c + 2, []).append(
                        lambda g=g: k_transposes(2, 4, 4 * g)
                    )
                    after.setdefault(c + 3, []).append(lambda g=g: v_copy(g))
                # q4-15 at the tail of block 0
                after.setdefault(27, []).append(lambda: norms(36, 42))
                qh = [None]

                def q_cast(lo, hi):
                    qh[0] = unit_cast(lo, hi)

                def q_transposes(a, b, base):
                    for j in range(a, b):
                        transpose_tile(qh[0], j, base + j, qt)

                after.setdefault(28, []).append(lambda: q_cast(36, 42))
                after.setdefault(28, []).append(lambda: q_transposes(0, 2, 4))
                after.setdefault(29, []).append(lambda: q_transposes(2, 6, 4))
                after.setdefault(30, []).append(lambda: norms(42, 48))
                after.setdefault(31, []).append(lambda: q_cast(42, 48))
                after.setdefault(31, []).append(lambda: q_transposes(0, 6, 10))

                # block 0 main loop: 1-chunk score tiles, software-pipelined
                # one chunk ahead so the PE never waits on ACT
                def st_mm(kk):
                    st = ps1.tile([P, QB], F32, tag="st", name=f"st0_{kk}")
                    nc.tensor.matmul(
                        st,
                        lhsT=kt[:, :, kk * P : (kk + 1) * P],
                        rhs=qt[:, :, 0:QB],
                        start=True,
                        stop=True,
                        perf_mode=DRow,
                    )
                    pt = pt_pool.tile([P, QB], F32R, tag="pt", name=f"pt0_{kk}")
                    nc.scalar.activation(pt, st, Exp, scale=SCALE)
                    return pt

                pts = st_mm(0)
                for kk in range(KC):
                    pt_next = st_mm(kk + 1) if kk + 1 < KC else None
                    for t in range(QT):
                        nc.tensor.matmul(
                            avs0[t],
                            lhsT=pts[:, t * P : (t + 1) * P],
                            rhs=va[:, kk, :],
                            start=(kk == 0),
                            stop=(kk == KC - 1),
                        )
                    pts = pt_next
                    for thunk in after.get(kk, ()):
                        thunk()
                epilogue(0, avs0)

            # ================= phase B: blocks 1-3 =================
            with tc.tile_pool(name="ps2", bufs=2, space="PSUM") as ps2:
                for blk in range(1, NB):
                    avs = new_avs(blk)
                    NG = KC // 2

                    def grp_mm(g, blk=blk):
                        st2 = ps2.tile(
                            [P, 2, QB], F32, tag="st2", name=f"st{blk}_{g}"
                        )
                        for j in range(2):
                            kk = 2 * g + j
                            nc.tensor.matmul(
                                st2[:, j, :],
                                lhsT=kt[:, :, kk * P : (kk + 1) * P],
                                rhs=qt[:, :, blk * QB : (blk + 1) * QB],
                                start=True,
                                stop=True,
                                perf_mode=DRow,
                            )
                        pt2 = pt_pool.tile(
                            [P, 2, QB], F32R, tag="pt2", name=f"pt{blk}_{g}"
                        )
                        nc.scalar.activation(pt2, st2, Exp, scale=SCALE)
                        return pt2

                    pts = grp_mm(0)
                    for g in range(NG):
                        pt_next = grp_mm(g + 1) if g + 1 < NG else None
                        for j in range(2):
                            kk = 2 * g + j
                            for t in range(QT):
                                nc.tensor.matmul(
                                    avs[t],
                                    lhsT=pts[:, j, t * P : (t + 1) * P],
                                    rhs=va[:, kk, :],
                                    start=(kk == 0),
                                    stop=(kk == KC - 1),
                                )
                        pts = pt_next
                    epilogue(blk, avs)

    nc.compile()
    return nc


_CACHED = {}


def _get_program():
    if "nc" not in _CACHED:
        _CACHED["nc"] = _build_program()
    return _CACHED["nc"]


def _get_runner():
    """Cached jitted shard_map executor (run_bass_via_pjrt rebuilds its jit
    wrapper on every call; caching it saves ~1-2s of retrace per invocation)."""
    if "runner" in _CACHED:
        return _CACHED["runner"]
    import jax
    from jax.sharding import Mesh, PartitionSpec
    from jax.experimental.shard_map import shard_map
    from concourse import bass2jax
    import concourse.mybir as _mb

    nc = _get_program()
    bass2jax.install_neuronx_cc_hook()

    partition_name = nc.partition_id_tensor.name if nc.partition_id_tensor else None
    in_names, out_names, out_avals, zero_outs = [], [], [], []
    for alloc in nc.m.functions[0].allocations:
        if not isinstance(alloc, _mb.MemoryLocationSet):
            continue
        name = alloc.memorylocations[0].name
        if alloc.kind == "ExternalInput":
            if name != partition_name:
                in_names.append(name)
        elif alloc.kind == "ExternalOutput":
            shape = tuple(alloc.tensor_shape)
            npdt = _mb.dt.np(alloc.dtype)
            out_names.append(name)
            out_avals.append(jax.core.ShapedArray(shape, npdt))
            zero_outs.append(np.zeros(shape, npdt))
    n_params = len(in_names)
    n_outs = len(out_names)
    all_names = in_names + out_names
    if partition_name is not None:
        all_names = all_names + [partition_name]
    donate = tuple(range(n_params, n_params + n_outs))

    def _body(*args):
        operands = list(args)
        if partition_name is not None:
            operands.append(bass2jax.partition_id_tensor())
        outs = bass2jax._bass_exec_p.bind(
            *operands,
            out_avals=tuple(out_avals),
            in_names=tuple(all_names),
            out_names=tuple(out_names),
            lowering_input_output_aliases=(),
            sim_require_finite=True,
            sim_require_nnan=True,
            nc=nc,
        )
        return tuple(outs)

    devices = jax.devices()[:N_CORES]
    mesh = Mesh(np.asarray(devices), ("core",))
    sharded = jax.jit(
        shard_map(
            _body,
            mesh=mesh,
            in_specs=(PartitionSpec("core"),) * (n_params + n_outs),
            out_specs=(PartitionSpec("core"),) * n_outs,
            check_rep=False,
        ),
        donate_argnums=donate,
        keep_unused=True,
    )

    def run(in_maps):
        concat_in = [
            np.concatenate([m[name] for m in in_maps], axis=0) for name in in_names
        ]
        concat_zeros = [
            np.zeros((N_CORES * z.shape[0], *z.shape[1:]), z.dtype) for z in zero_outs
        ]
        out_arrs = sharded(*concat_in, *concat_zeros)
        return [
            {
                name: np.asarray(out_arrs[i]).reshape(N_CORES, *out_avals[i].shape)[c]
                for i, name in enumerate(out_names)
            }
            for c in range(N_CORES)
        ]

    _CACHED["runner"] = run
    return run


def _make_in_maps(query, key, value):
    in_maps = []
    for c in range(N_CORES):
        b = c // (N_CORES // B)
        qs = (c % (N_CORES // B)) * NQ
        in_maps.append(
            {
                "q": np.ascontiguousarray(query[b, qs : qs + NQ], dtype=np.float32),
                "k": np.ascontiguousarray(key[b], dtype=np.float32),
                "v": np.ascontiguousarray(value[b], dtype=np.float32),
            }
        )
    return in_maps


def _gather(results):
    out = np.empty((B, NQ_FULL, D), dtype=np.float32)
    for c in range(N_CORES):
        b = c // (N_CORES // B)
        qs = (c % (N_CORES // B)) * NQ
        out[b, qs : qs + NQ] = results[c]["o"]
    return out


def run_sharded(query, key, value, trace=False):
    """Returns (out, BassKernelResults). trace=True goes through the
    profiling path; the fast path uses the cached jitted executor."""
    in_maps = _make_in_maps(query, key, value)
    if trace:
        nc = _get_program()
        res = run_bass_kernel_spmd(
            nc, in_maps, core_ids=list(range(N_CORES)), trace=True
        )
        return _gather(res.results), res
    run = _get_runner()
    return _gather(run(in_maps)), None


def kernel(query, key, value):
    query = np.asarray(query)
    key = np.asarray(key)
    value = np.asarray(value)
    try:
        out, _ = run_sharded(query, key, value)
    except Exception:
        # fall back to the framework executor if the cached-runner fast
        # path hits an incompatibility
        nc = _get_program()
        in_maps = _make_in_maps(query, key, value)
        res = run_bass_kernel_spmd(nc, in_maps, core_ids=list(range(N_CORES)))
        out = _gather(res.results)
    return out
